# revision 15
# baseline (speedup 1.0000x reference)
"""Trainium2 Bass kernel for nn_DepthToGaussian.

Data-parallel over batch B=8 across 8 NeuronCores (1 sample/core).
BatchNorm batch statistics are exchanged with tiny AllReduce collectives.
FPS (farthest point sampling) runs on-device, bit-exact with the CPU/XLA
reference so the selected index sequence matches exactly.
"""

import os
import sys
from contextlib import ExitStack

import numpy as np

for _p in ("/opt/trn_rl_repo", "/root/.axon_site/_ro/trn_rl_repo"):
    if os.path.isdir(_p) and _p not in sys.path:
        sys.path.insert(0, _p)

import concourse.bacc as bacc
import concourse.bass as bass
import concourse.mybir as mybir
from concourse.tile import TileContext
from concourse.bass_utils import run_bass_kernel_spmd

F32 = mybir.dt.float32
I32 = mybir.dt.int32
AX = mybir.AxisListType
OP = mybir.AluOpType
AF = mybir.ActivationFunctionType

N_CORES = 8
H = W = 512
HF = WF = 128
NPTS = HF * WF          # 16384
NG = 2048               # gaussians sampled
PD = 35                 # param dim
FEAT = 128
EPS = 1e-5

FPS_ITERS = int(os.environ.get("FPS_ITERS", NG))
FPS_UNROLL = int(os.environ.get("FPS_UNROLL", "16"))
SKIP_CONV = os.environ.get("SKIP_CONV", "0") == "1"
SKIP_FPS = os.environ.get("SKIP_FPS", "0") == "1"
FPS_LEVEL = int(os.environ.get("FPS_LEVEL", "9"))

_CACHED = {}


# ---------------------------------------------------------------- host math
def _points_host(depth, K):
    """Bit-exact replication of the reference backproject + 4x4 avg pool
    (verified bitwise-identical to jax-CPU)."""
    B = depth.shape[0]
    d = depth[:, 0]
    u = np.arange(W, dtype=np.float32)[None, None, :]
    v = np.arange(H, dtype=np.float32)[None, :, None]
    fx = K[:, 0, 0][:, None, None]
    fy = K[:, 1, 1][:, None, None]
    cx = K[:, 0, 2][:, None, None]
    cy = K[:, 1, 2][:, None, None]
    x = (u - cx) * d / fx
    y = (v - cy) * d / fy
    pts = np.stack([x, y, d], axis=-1)  # B,H,W,3
    pd = pts.transpose(0, 3, 1, 2).reshape(B, 3, HF, 4, WF, 4)
    pd = pd.mean((3, 5), dtype=np.float32).astype(np.float32)
    return pd.transpose(0, 2, 3, 1).reshape(B, NPTS, 3)


def _pack_w0(w0):
    """w0 [128,4,7,7] -> per-shift lhsT [16][16,128].
    partition = (ry*2+rx)*4 + ch ; shift s = (sy+2)*4 + (sx+2)."""
    out = np.zeros((16, 16, 128), np.float32)
    for sy in range(-2, 2):
        for sx in range(-2, 2):
            s = (sy + 2) * 4 + (sx + 2)
            for ry in range(2):
                for rx in range(2):
                    ky = 2 * sy + ry + 3
                    kx = 2 * sx + rx + 3
                    if 0 <= ky <= 6 and 0 <= kx <= 6:
                        for ch in range(4):
                            out[s, (ry * 2 + rx) * 4 + ch, :] = w0[:, ch, ky, kx]
    return out


def _pack_3x3(w):
    """w [128,128,3,3] -> [9,128,128] lhsT per tap (ky*3+kx): [ic, oc]."""
    return np.ascontiguousarray(w.transpose(2, 3, 1, 0).reshape(9, 128, 128))


# ---------------------------------------------------------------- bass build
def _build():
    nc = bacc.Bacc("TRN2", target_bir_lowering=False, debug=False,
                   num_devices=N_CORES)
    ctx = ExitStack()

    def din(name, shape):
        return nc.dram_tensor(name, shape, F32, kind="ExternalInput")

    stacked_i = din("stacked", [16, 260, 260])
    px_i = din("px", [128, 128])
    py_i = din("py", [128, 128])
    pz_i = din("pz", [128, 128])
    iota_i = din("iota", [128, 128])
    c0b_i = din("c0b", [128, 3])
    c0r_i = din("c0r", [1, 3])
    ident_i = din("ident", [128, 128])
    ones1_i = din("ones1", [1, 128])
    ones128_i = din("ones128", [128, 1])
    w0t_i = din("w0t", [16, 16 * 128])
    w1t_i = din("w1t", [128, 9 * 128])
    rwt_i = din("rwt", [8, 128, 9 * 128])
    hw0_i = din("hw0t", [128, 128])
    hw1_i = din("hw1t", [128, PD])
    hb0_i = din("hb0", [128, 1])
    hb1_i = din("hb1", [PD, 1])
    pts3_i = din("pts3", [NPTS, 3])
    g_i = din("g_all", [10, 128, 1])
    be_i = din("be_all", [10, 128, 1])

    out_t = nc.dram_tensor("out", [NG, 38], F32, kind="ExternalOutput")

    conv0raw = nc.dram_tensor("conv0raw", [128, 256 * 256], F32)
    h2stage = nc.dram_tensor("h2stage", [128, NPTS], F32)
    pflat_d = nc.dram_tensor("pflat_d", [NPTS, PD], F32)
    idx_d = nc.dram_tensor("idx_d", [NG], F32, kind="ExternalOutput")
    cc_in = [nc.dram_tensor(f"cc_in{i}", [128, 2], F32) for i in range(10)]
    cc_out = [nc.dram_tensor(f"cc_out{i}", [128, 2], F32, addr_space="Shared")
              for i in range(10)]
    GROUPS = [list(range(N_CORES))]

    with TileContext(nc) as tc:
        with ctx:
            pool = ctx.enter_context(tc.tile_pool(name="main", bufs=1))
            wpool = ctx.enter_context(tc.tile_pool(name="wts", bufs=2))
            spool = ctx.enter_context(tc.tile_pool(name="stream", bufs=2))
            psum = ctx.enter_context(tc.tile_pool(name="ps", bufs=2, space="PSUM"))
            psum1 = ctx.enter_context(tc.tile_pool(name="ps1", bufs=2, space="PSUM"))
            fstack = ExitStack()
            fpool = fstack.enter_context(tc.tile_pool(name="fps", bufs=1))

            # ---------------- shared constants
            ident = pool.tile([128, 128], F32, name="ident")
            nc.sync.dma_start(out=ident[:], in_=ident_i[:])
            ones1 = pool.tile([1, 128], F32, name="ones1")
            nc.sync.dma_start(out=ones1[:], in_=ones1_i[:])
            ones128 = pool.tile([128, 1], F32, name="ones128")
            nc.sync.dma_start(out=ones128[:], in_=ones128_i[:])

            # ============================================================
            # Phase F: farthest point sampling (bit-exact with reference)
            # ============================================================
            if not SKIP_FPS:
                px = fpool.tile([128, 128], F32, name="px")
                py = fpool.tile([128, 128], F32, name="py")
                pz = fpool.tile([128, 128], F32, name="pz")
                iot = fpool.tile([128, 128], F32, name="iot")
                D = fpool.tile([128, 128], F32, name="D")
                cand = fpool.tile([128, 128], F32, name="cand")
                t1 = fpool.tile([128, 128], F32, name="t1")
                t2 = fpool.tile([128, 128], F32, name="t2")
                t3 = fpool.tile([128, 128], F32, name="t3")
                prod = pool.tile([128, 128], F32, name="prodt")
                rowmax = fpool.tile([128, 1], F32, name="rowmax")
                rowmin = fpool.tile([128, 1], F32, name="rowmin")
                rds = fpool.tile([128, 3], F32, name="rds")
                cb = fpool.tile([128, 3], F32, name="cbt")
                gmb = fpool.tile([128, 1], F32, name="gmb")
                gib = fpool.tile([128, 1], F32, name="gib")
                gm = fpool.tile([1, 1], F32, name="gm")
                gi = fpool.tile([1, 1], F32, name="gi")
                coords = fpool.tile([1, 3], F32, name="coords")
                idxl = pool.tile([1, NG], F32, name="idxl")

                nc.sync.dma_start(out=px[:], in_=px_i[:])
                nc.sync.dma_start(out=py[:], in_=py_i[:])
                nc.sync.dma_start(out=pz[:], in_=pz_i[:])
                nc.sync.dma_start(out=iot[:], in_=iota_i[:])
                nc.sync.dma_start(out=cb[:], in_=c0b_i[:])
                nc.sync.dma_start(out=coords[:], in_=c0r_i[:])
                nc.vector.memset(D[:], 3.4e38)
                nc.vector.memset(gi[:], 0.0)
                nc.vector.memset(idxl[:], 0.0)

                def fps_body(i):
                    # record current selection (pre-update farthest)
                    nc.scalar.copy(idxl[0:1, bass.ds(i, 1)], gi[:])
                    if FPS_LEVEL < 1:
                        return
                    # dist = (x-cx)^2 + (y-cy)^2 + (z-cz)^2  (exact order)
                    nc.vector.tensor_scalar(t1[:], px[:], cb[:, 0:1], None, OP.subtract)
                    nc.vector.tensor_tensor(t1[:], t1[:], t1[:], OP.mult)
                    nc.vector.tensor_scalar(t2[:], py[:], cb[:, 1:2], None, OP.subtract)
                    nc.vector.tensor_tensor(t2[:], t2[:], t2[:], OP.mult)
                    nc.vector.tensor_scalar(t3[:], pz[:], cb[:, 2:3], None, OP.subtract)
                    nc.vector.tensor_tensor(t3[:], t3[:], t3[:], OP.mult)
                    nc.vector.tensor_tensor(t1[:], t1[:], t2[:], OP.add)
                    nc.vector.tensor_tensor(t1[:], t1[:], t3[:], OP.add)
                    # D = min(D, dist); rowmax = max over row
                    if os.environ.get("NO_TTR", "1") == "1":
                        nc.vector.tensor_tensor(D[:], D[:], t1[:], OP.min)
                        nc.vector.tensor_reduce(rowmax[:], D[:], axis=AX.X, op=OP.max)
                    else:
                        nc.vector.tensor_tensor_reduce(
                            out=D[:], in0=D[:], in1=t1[:], scale=1.0, scalar=-3.4e38,
                            op0=OP.min, op1=OP.max, accum_out=rowmax[:])
                    if FPS_LEVEL < 2:
                        return
                    # global max
                    trp = psum1.tile([1, 128], F32, tag="trp")
                    nc.tensor.transpose(out=trp[:], in_=rowmax[:], identity=ident[:])
                    nc.vector.tensor_reduce(gm[:], trp[:], axis=AX.X, op=OP.max)
                    if FPS_LEVEL < 3:
                        return
                    bc = psum1.tile([128, 1], F32, tag="bcst")
                    nc.tensor.matmul(bc[:], ones1[:], gm[:], start=True, stop=True)
                    nc.scalar.copy(gmb[:], bc[:])
                    # first index achieving the max
                    nc.vector.tensor_scalar(cand[:], D[:], gmb[:], 1e9, OP.is_lt, OP.mult)
                    if os.environ.get("NO_TTR", "1") == "1":
                        nc.vector.tensor_tensor(cand[:], cand[:], iot[:], OP.add)
                        nc.vector.tensor_reduce(rowmin[:], cand[:], axis=AX.X, op=OP.min)
                    else:
                        nc.vector.tensor_tensor_reduce(
                            out=cand[:], in0=cand[:], in1=iot[:], scale=1.0, scalar=3.4e38,
                            op0=OP.add, op1=OP.min, accum_out=rowmin[:])
                    if FPS_LEVEL < 4:
                        return
                    trp2 = psum1.tile([1, 128], F32, tag="trp")
                    nc.tensor.transpose(out=trp2[:], in_=rowmin[:], identity=ident[:])
                    nc.vector.tensor_reduce(gi[:], trp2[:], axis=AX.X, op=OP.min)
                    bc2 = psum1.tile([128, 1], F32, tag="bcst")
                    nc.tensor.matmul(bc2[:], ones1[:], gi[:], start=True, stop=True)
                    nc.scalar.copy(gib[:], bc2[:])
                    if FPS_LEVEL < 5:
                        return
                    # gather selected point coords via one-hot mask
                    nc.vector.tensor_scalar(prod[:], iot[:], gib[:], None, OP.is_equal)
                    if os.environ.get("NO_TTR", "1") == "1":
                        nc.vector.tensor_tensor(t1[:], prod[:], px[:], OP.mult)
                        nc.vector.tensor_reduce(rds[:, 0:1], t1[:], axis=AX.X, op=OP.add)
                        nc.vector.tensor_tensor(t2[:], prod[:], py[:], OP.mult)
                        nc.vector.tensor_reduce(rds[:, 1:2], t2[:], axis=AX.X, op=OP.add)
                        nc.vector.tensor_tensor(t3[:], prod[:], pz[:], OP.mult)
                        nc.vector.tensor_reduce(rds[:, 2:3], t3[:], axis=AX.X, op=OP.add)
                    else:
                        nc.vector.tensor_tensor_reduce(
                            out=t1[:], in0=prod[:], in1=px[:], scale=1.0, scalar=0.0,
                            op0=OP.mult, op1=OP.add, accum_out=rds[:, 0:1])
                        nc.vector.tensor_tensor_reduce(
                            out=t2[:], in0=prod[:], in1=py[:], scale=1.0, scalar=0.0,
                            op0=OP.mult, op1=OP.add, accum_out=rds[:, 1:2])
                        nc.vector.tensor_tensor_reduce(
                            out=t3[:], in0=prod[:], in1=pz[:], scale=1.0, scalar=0.0,
                            op0=OP.mult, op1=OP.add, accum_out=rds[:, 2:3])
                    cp = psum1.tile([1, 3], F32, tag="bcst")
                    nc.tensor.matmul(cp[:], ones128[:], rds[:], start=True, stop=True)
                    nc.scalar.copy(coords[:], cp[:])
                    cbp = psum1.tile([128, 3], F32, tag="bcst")
                    nc.tensor.matmul(cbp[:], ones1[:], coords[:], start=True, stop=True)
                    nc.scalar.copy(cb[:], cbp[:])

                if FPS_ITERS >= FPS_UNROLL and FPS_ITERS % FPS_UNROLL == 0:
                    tc.For_i_unrolled(0, FPS_ITERS, 1, fps_body,
                                      max_unroll=FPS_UNROLL)
                else:
                    for i in range(FPS_ITERS):
                        fps_body(i)

                nc.sync.dma_start(out=idx_d.ap().unsqueeze(0), in_=idxl[:])
                fstack.close()

            # ============================================================
            # Conv encoder
            # ============================================================
            stats_s = pool.tile([128, 128], F32, name="stats_s")
            stats_q = pool.tile([128, 128], F32, name="stats_q")
            scl = pool.tile([128, 1], F32, name="scl")
            bia = pool.tile([128, 1], F32, name="bia")
            st2 = pool.tile([128, 2], F32, name="st2")
            sq_scratch = pool.tile([128, 512], F32, name="sq_scratch")
            gl = pool.tile([128, 1], F32, name="gl")
            bel = pool.tile([128, 1], F32, name="bel")
            tta = pool.tile([128, 1], F32, name="tta")
            ttb = pool.tile([128, 1], F32, name="ttb")

            def finish_stats(layer, count):
                """stats_s/q chunk columns (nchunks used) -> scl/bia via AllReduce."""
                nch = count // 512
                nc.vector.tensor_reduce(st2[:, 0:1], stats_s[:, 0:nch],
                                        axis=AX.X, op=OP.add)
                nc.vector.tensor_reduce(st2[:, 1:2], stats_q[:, 0:nch],
                                        axis=AX.X, op=OP.add)
                nc.sync.dma_start(out=cc_in[layer][:], in_=st2[:])
                nc.gpsimd.collective_compute(
                    "AllReduce", OP.add, replica_groups=GROUPS,
                    ins=[cc_in[layer][:]], outs=[cc_out[layer][:]])
                nc.sync.dma_start(out=st2[:], in_=cc_out[layer][:])
                inv = 1.0 / (N_CORES * count)
                # mean, E[x^2]
                nc.vector.tensor_scalar(tta[:], st2[:, 0:1], inv, None, OP.mult)
                nc.vector.tensor_scalar(ttb[:], st2[:, 1:2], inv, None, OP.mult)
                # var = E[x^2] - mean^2  (+eps)
                nc.vector.tensor_tensor(scl[:], tta[:], tta[:], OP.mult)
                nc.vector.tensor_tensor(ttb[:], ttb[:], scl[:], OP.subtract)
                nc.vector.tensor_scalar(ttb[:], ttb[:], EPS, None, OP.add)
                # invstd
                nc.scalar.sqrt(ttb[:], ttb[:])
                nc.sync.dma_start(out=gl[:], in_=g_i[layer])
                nc.sync.dma_start(out=bel[:], in_=be_i[layer])
                nc.vector.reciprocal(ttb[:], ttb[:])
                nc.vector.tensor_tensor(scl[:], gl[:], ttb[:], OP.mult)
                # bias = be - mean*scale
                nc.vector.tensor_tensor(tta[:], tta[:], scl[:], OP.mult)
                nc.vector.tensor_tensor(bia[:], bel[:], tta[:], OP.subtract)

            if not SKIP_CONV:
                # -------- conv0: 7x7 s2, 4->128, phase-stacked K=16
                w0 = wpool.tile([16, 16 * 128], F32, tag="w")
                nc.sync.dma_start(out=w0[:], in_=w0t_i[:])
                c0stack = ExitStack()
                c0pool = c0stack.enter_context(tc.tile_pool(name="c0p", bufs=2))
                for half in range(8):
                    stk = c0pool.tile([16, 36, 260], F32, tag="stk")
                    phr0 = 32 * half - 2  # phase row of tile row 0
                    nc.sync.dma_start(out=stk[:],
                                      in_=stacked_i.ap()[:, 32 * half:32 * half + 36, :])
                    c_lo = 16 * half
                    for c in range(c_lo, c_lo + 16):
                        ps = psum.tile([128, 512], F32, tag="mm")
                        for s in range(16):
                            sy, sx = s // 4 - 2, s % 4 - 2
                            ro = 2 * c + sy - phr0
                            rhs = stk[:, bass.ds(ro, 2, 1),
                                      bass.ds(sx + 2, 256, 1)]
                            nc.tensor.matmul(ps[:], w0[:, bass.ts(s, 128)],
                                             rhs, start=(s == 0), stop=(s == 15))
                        tmp = spool.tile([128, 512], F32, tag="c0tmp")
                        nc.scalar.activation(tmp[:], ps[:], AF.Copy,
                                             accum_out=stats_s[:, c:c + 1])
                        nc.scalar.activation(sq_scratch[:], ps[:], AF.Square,
                                             accum_out=stats_q[:, c:c + 1])
                        nc.sync.dma_start(out=conv0raw[:, bass.ts(c, 512)],
                                          in_=tmp[:])
                c0stack.close()
                finish_stats(0, 256 * 256)

                # -------- conv1: 3x3 s2 128->128, banded from conv0raw
                w1 = wpool.tile([128, 9 * 128], F32, tag="w")
                nc.sync.dma_start(out=w1[:], in_=w1t_i[:])
                cpool = ctx.enter_context(tc.tile_pool(name="cbody", bufs=1))
                xbuf = cpool.tile([128, 130, 130], F32, name="xbuf")
                h1buf = cpool.tile([128, 130, 130], F32, name="h1buf")
                nc.vector.memset(xbuf[:], 0.0)
                for b in range(32):
                    band = spool.tile([128, 9, 258], F32, tag="band")
                    nc.vector.memset(band[:], 0.0)
                    r0 = 8 * b - 1
                    rv0 = max(0, r0)
                    rv1 = min(255, r0 + 8)
                    nrow = rv1 - rv0 + 1
                    bvalid = band[:, bass.ds(rv0 - r0, nrow, 1), bass.ds(1, 256, 1)]
                    nc.sync.dma_start(
                        out=bvalid,
                        in_=conv0raw.ap().rearrange("p (r c) -> p r c", c=256)[:, rv0:rv0 + nrow, :])
                    # bn0 + gelu in place on valid region
                    nc.scalar.activation(bvalid, bvalid, AF.Gelu,
                                         bias=bia[:], scale=scl[:])
                    ps = psum.tile([128, 512], F32, tag="mm")
                    for t in range(9):
                        ky, kx = t // 3, t % 3
                        rhs = band[:, bass.ds(ky, 4, 2), bass.ds(kx, 128, 2)]
                        nc.tensor.matmul(ps[:], w1[:, bass.ts(t, 128)],
                                         rhs, start=(t == 0), stop=(t == 8))
                    # raw into xbuf interior rows 4b..4b+3
                    dst = xbuf[:, bass.ds(1 + 4 * b, 4, 1), bass.ds(1, 128, 1)]
                    nc.scalar.activation(dst, ps[:], AF.Copy,
                                         accum_out=stats_s[:, b:b + 1])
                    nc.scalar.activation(sq_scratch[:], ps[:], AF.Square,
                                         accum_out=stats_q[:, b:b + 1])
                finish_stats(1, NPTS)
                xint = xbuf[:, bass.ds(1, 128, 1), bass.ds(1, 128, 1)]
                nc.scalar.activation(xint, xint, AF.Gelu, bias=bia[:], scale=scl[:])

                # -------- 4 residual blocks
                for blk in range(4):
                    for sub in range(2):
                        layer = 2 + blk * 2 + sub
                        wt = wpool.tile([128, 9 * 128], F32, tag="w")
                        nc.sync.dma_start(out=wt[:], in_=rwt_i[blk * 2 + sub])
                        src = xbuf if sub == 0 else h1buf
                        if sub == 0:
                            nc.vector.memset(h1buf[:], 0.0)
                        for c in range(32):
                            ps = psum.tile([128, 512], F32, tag="mm")
                            for t in range(9):
                                ky, kx = t // 3, t % 3
                                rhs = src[:, bass.ds(4 * c + ky, 4, 1),
                                          bass.ds(kx, 128, 1)]
                                nc.tensor.matmul(ps[:],
                                                 wt[:, bass.ts(t, 128)], rhs,
                                                 start=(t == 0), stop=(t == 8))
                            if sub == 0:
                                dst = h1buf[:, bass.ds(1 + 4 * c, 4, 1),
                                            bass.ds(1, 128, 1)]
                                nc.scalar.activation(dst, ps[:], AF.Copy,
                                                     accum_out=stats_s[:, c:c + 1])
                                nc.scalar.activation(sq_scratch[:], ps[:], AF.Square,
                                                     accum_out=stats_q[:, c:c + 1])
                            else:
                                stg = spool.tile([128, 512], F32, tag="h2st")
                                nc.scalar.activation(stg[:], ps[:], AF.Copy,
                                                     accum_out=stats_s[:, c:c + 1])
                                nc.scalar.activation(sq_scratch[:], ps[:], AF.Square,
                                                     accum_out=stats_q[:, c:c + 1])
                                nc.sync.dma_start(out=h2stage[:, bass.ts(c, 512)],
                                                  in_=stg[:])
                        finish_stats(layer, NPTS)
                        if sub == 0:
                            h1i = h1buf[:, bass.ds(1, 128, 1), bass.ds(1, 128, 1)]
                            nc.scalar.activation(h1i, h1i, AF.Gelu,
                                                 bias=bia[:], scale=scl[:])
                        else:
                            # y = gelu(h2*scl + x + bia) written back into xbuf
                            for c in range(32):
                                h2c = spool.tile([128, 512], F32, tag="h2ld")
                                nc.sync.dma_start(out=h2c[:],
                                                  in_=h2stage[:, bass.ts(c, 512)])
                                xi = xbuf[:, bass.ds(1 + 4 * c, 4, 1),
                                          bass.ds(1, 128, 1)]
                                tmp2 = spool.tile([128, 4, 128], F32, tag="yt")
                                nc.vector.scalar_tensor_tensor(
                                    out=tmp2[:], in0=h2c[:].rearrange("p (a b) -> p a b", b=128),
                                    scalar=scl[:], in1=xi,
                                    op0=OP.mult, op1=OP.add)
                                nc.scalar.activation(xi, tmp2[:], AF.Gelu, bias=bia[:])

                # -------- head: 1x1 convs 128->128 gelu -> 35
                hw0 = wpool.tile([128, 128], F32, name="hw0")
                nc.sync.dma_start(out=hw0[:], in_=hw0_i[:])
                hw1 = wpool.tile([128, PD], F32, name="hw1")
                nc.sync.dma_start(out=hw1[:], in_=hw1_i[:])
                hb0 = pool.tile([128, 1], F32, name="hb0")
                nc.sync.dma_start(out=hb0[:], in_=hb0_i[:])
                hb1 = pool.tile([PD, 1], F32, name="hb1")
                nc.sync.dma_start(out=hb1[:], in_=hb1_i[:])
                for c in range(32):
                    xi = xbuf[:, bass.ds(1 + 4 * c, 4, 1), bass.ds(1, 128, 1)]
                    ps = psum.tile([128, 512], F32, tag="mm")
                    nc.tensor.matmul(ps[:], hw0[:], xi, start=True, stop=True)
                    ht = spool.tile([128, 512], F32, tag="ht")
                    nc.scalar.activation(ht[:], ps[:], AF.Gelu, bias=hb0[:])
                    ps2 = psum1.tile([PD, 512], F32, tag="hps2")
                    nc.tensor.matmul(ps2[:], hw1[:], ht[:], start=True, stop=True)
                    prt = spool.tile([PD, 512], F32, tag="prt")
                    nc.scalar.activation(prt[:], ps2[:], AF.Identity, bias=hb1[:])
                    # store transposed: pflat[pix, param]
                    nc.sync.dma_start(
                        out=pflat_d.ap().rearrange("n p -> p n")[:, bass.ds(c * 512, 512, 1)],
                        in_=prt[:])

            # ============================================================
            # Gather + finalize
            # ============================================================
            if not SKIP_FPS and not SKIP_CONV:
                for j in range(16):
                    idxf = spool.tile([128, 1], F32, tag="idxf")
                    nc.sync.dma_start(out=idxf[:],
                                      in_=idx_d.ap().rearrange("(a b) -> a b", b=128)[j].unsqueeze(1))
                    idxi = spool.tile([128, 1], I32, tag="idxi")
                    nc.vector.tensor_copy(idxi[:], idxf[:])
                    pg = spool.tile([128, PD], F32, tag="pg")
                    nc.gpsimd.indirect_dma_start(
                        out=pg[:], out_offset=None, in_=pflat_d[:],
                        in_offset=bass.IndirectOffsetOnAxis(ap=idxi[:, 0:1], axis=0))
                    ptg = spool.tile([128, 3], F32, tag="ptg")
                    nc.gpsimd.indirect_dma_start(
                        out=ptg[:], out_offset=None, in_=pts3_i[:],
                        in_offset=bass.IndirectOffsetOnAxis(ap=idxi[:, 0:1], axis=0))
                    ot = spool.tile([128, 38], F32, tag="ot")
                    nc.vector.tensor_copy(ot[:, 0:3], ptg[:])
                    # scale = softplus = ln(1 + exp(x))
                    spt = spool.tile([128, 3], F32, tag="spt")
                    nc.scalar.activation(spt[:], pg[:, 0:3], AF.Exp)
                    nc.vector.tensor_scalar(spt[:], spt[:], 1.0, None, OP.add)
                    nc.scalar.activation(ot[:, 3:6], spt[:], AF.Ln)
                    # rot normalize
                    rsq = spool.tile([128, 4], F32, tag="rsq")
                    nc.vector.tensor_tensor(rsq[:], pg[:, 3:7], pg[:, 3:7], OP.mult)
                    rn = spool.tile([128, 1], F32, tag="rn")
                    nc.vector.tensor_reduce(rn[:], rsq[:].unsqueeze(1),
                                            axis=AX.X, op=OP.add)
                    nc.scalar.sqrt(rn[:], rn[:])
                    nc.vector.tensor_scalar(rn[:], rn[:], 1e-12, None, OP.max)
                    nc.vector.reciprocal(rn[:], rn[:])
                    nc.vector.tensor_scalar(ot[:, 6:10], pg[:, 3:7], rn[:],
                                            None, OP.mult)
                    nc.scalar.activation(ot[:, 10:11], pg[:, 7:8], AF.Sigmoid)
                    nc.vector.tensor_copy(ot[:, 11:38], pg[:, 8:35])
                    nc.sync.dma_start(out=out_t.ap().rearrange("(a b) c -> a b c", b=128)[j],
                                      in_=ot[:])

    nc.finalize()
    return nc


# ---------------------------------------------------------------- entry
def kernel(rgb, depth, intrinsics, params):
    rgb = np.asarray(rgb, np.float32)
    depth = np.asarray(depth, np.float32)
    K = np.asarray(intrinsics, np.float32)
    B = rgb.shape[0]
    points = _points_host(depth, K)  # [B, 16384, 3]

    if "nc" not in _CACHED:
        _CACHED["nc"] = _build()
    nc = _CACHED["nc"]

    p = params
    w0t = _pack_w0(np.asarray(p["w0"], np.float32))
    w0t = np.ascontiguousarray(w0t.transpose(1, 0, 2).reshape(16, 16 * 128))
    w1t = _pack_3x3(np.asarray(p["w1"], np.float32))
    w1t = np.ascontiguousarray(w1t.transpose(1, 0, 2).reshape(128, 9 * 128))
    rwt = np.stack(
        [np.ascontiguousarray(
            _pack_3x3(np.asarray(rb[k], np.float32))
            .transpose(1, 0, 2).reshape(128, 9 * 128))
         for rb in p["res"] for k in ("w1", "w2")])  # [8,128,9*128]
    g_all = np.stack(
        [np.asarray(p["g0"], np.float32), np.asarray(p["g1"], np.float32)]
        + [np.asarray(rb[k], np.float32) for rb in p["res"] for k in ("g1", "g2")]
    ).reshape(10, 128, 1)
    be_all = np.stack(
        [np.asarray(p["be0"], np.float32), np.asarray(p["be1"], np.float32)]
        + [np.asarray(rb[k], np.float32) for rb in p["res"] for k in ("be1", "be2")]
    ).reshape(10, 128, 1)
    hw0t = np.ascontiguousarray(np.asarray(p["hw0"], np.float32)[:, :, 0, 0].T)
    hw1t = np.ascontiguousarray(np.asarray(p["hw1"], np.float32)[:, :, 0, 0].T)
    hb0 = np.asarray(p["hb0"], np.float32).reshape(128, 1)
    hb1 = np.asarray(p["hb1"], np.float32).reshape(PD, 1)

    iota = (np.arange(128, dtype=np.float32)[:, None] * 128
            + np.arange(128, dtype=np.float32)[None, :])
    ident = np.eye(128, dtype=np.float32)
    ones1 = np.ones((1, 128), np.float32)
    ones128 = np.ones((128, 1), np.float32)

    u4 = np.concatenate([rgb, depth], axis=1)  # [B,4,H,W]
    stacked_all = np.zeros((B, 16, 260, 260), np.float32)
    for ry in range(2):
        for rx in range(2):
            p0 = (ry * 2 + rx) * 4
            stacked_all[:, p0:p0 + 4, 2:258, 2:258] = u4[:, :, ry::2, rx::2]
    in_maps = []
    for b in range(B):
        pts = points[b]
        in_maps.append({
            "stacked": stacked_all[b],
            "px": np.ascontiguousarray(pts[:, 0].reshape(128, 128)),
            "py": np.ascontiguousarray(pts[:, 1].reshape(128, 128)),
            "pz": np.ascontiguousarray(pts[:, 2].reshape(128, 128)),
            "pts3": np.ascontiguousarray(pts), "iota": iota,
            "c0b": np.tile(pts[0:1], (128, 1)),
            "c0r": pts[0:1].copy(), "ident": ident, "ones1": ones1,
            "ones128": ones128,
            "w0t": w0t, "w1t": w1t, "rwt": rwt,
            "hw0t": hw0t, "hw1t": hw1t, "hb0": hb0, "hb1": hb1,
            "g_all": g_all, "be_all": be_all,
        })

    res = run_bass_kernel_spmd(nc, in_maps, list(range(N_CORES)))
    out = np.stack([res.results[c]["out"] for c in range(B)])
    return out.astype(np.float32)


if __name__ == "__main__":
    pass
